# revision 80
# baseline (speedup 1.0000x reference)
"""Distributed Trainium2 kernel for relative-position causal attention.

N=M=2048, B=1, D=1024, H=16, DQK=DV=64, OFFSET=0.
2 heads per core on 8 NeuronCores.

v2 structure (vs baseline):
  - inputs streamed column-chunked (1MB contiguous DMAs); projections
    interleave with attention so pair 0 starts ~10us in
  - scheme C softmax: exp(plr) and exp(content) separately on ACT
    (the PSUM drain IS the exp), position gathered post-exp as bf16 via
    the diagonal skew DMA, P = expC*expPos fused with the row-sum via
    scalar_tensor_tensor on DVE (one pass, accum_out = l)
  - logit matmuls head-paired: q/k/pe slices at partitions 0-63 vs
    64-127 auto-derive tile_position row groups -> concurrent on PE
  - ctx matmuls col-paired across the two heads (out partitions 0-63 /
    64-127)
  - transposes via diag(1/l) matmul (normalize+transpose fused), psT
    packs 4 P^T tiles per PSUM bank, single cast per 4 tiles
  - tail: two AllToAlls regrouped as pairs 0-3 / 4-7 (A2A#0 fires
    mid-kernel and its transfer hides under pairs 4-7); out-projection
    runs twice -- pass A executes during A2A#1's rendezvous (doubling
    as the HAM warm-keeper), pass B is the only serial tail; the host
    picks rows [0:256] on cores 0-3 and [256:512] on cores 4-7.
  - tile_wait_until pins collective-gated work at the end of the static
    schedule; queue split: sync ring = input chunks + skews + ships,
    scalar ring = compute only, gpsimd ring = weights + collectives +
    pass-A DMA.
"""

import sys

sys.path.insert(0, "/opt/trn_rl_repo")

import numpy as np
import ml_dtypes

from concourse import bass, bacc, tile, mybir
from concourse.ap import AP
from concourse.bass_utils import run_bass_kernel_spmd

N, M, D, H, DQK, DV = 2048, 2048, 1024, 16, 64, 64
RP = 2048
NCORES = 8
NB = N // 128
KT = D // 128          # 8 contraction k-tiles
F2 = RP + 128
CW = 256               # streamed input chunk width
NCH = N // CW          # 8 chunks per input tensor

BF = mybir.dt.bfloat16
F32 = mybir.dt.float32
RG = [list(range(NCORES))]
_cache = {}


def _build():
    nc = bacc.Bacc("TRN2", target_bir_lowering=False, debug=False,
                   num_devices=NCORES)
    ACTF = mybir.ActivationFunctionType
    MUL = mybir.AluOpType.mult

    # chunked inputs: [128, NCH*KT*CW], chunk c contiguous as [128,KT,CW]
    xqS = nc.dram_tensor("xqS", [128, NCH * KT * CW], BF, kind="ExternalInput")
    xkvS = nc.dram_tensor("xkvS", [128, NCH * KT * CW], BF,
                          kind="ExternalInput")
    scS = nc.dram_tensor("scS", [128, NCH * KT * CW], BF, kind="ExternalInput")
    wqT = nc.dram_tensor("wqT", [128, KT * 128], BF, kind="ExternalInput")
    wkT = nc.dram_tensor("wkT", [128, KT * 128], BF, kind="ExternalInput")
    wvT = nc.dram_tensor("wvT", [128, KT * 128], BF, kind="ExternalInput")
    fpeT = nc.dram_tensor("fpeT", [128, KT * 128], BF, kind="ExternalInput")
    woT = nc.dram_tensor("woT", [128, KT * D], BF, kind="ExternalInput")
    identc = nc.dram_tensor("identc", [128, 128], BF, kind="ExternalInput")
    out_ext = nc.dram_tensor("out", [512, D], F32, kind="ExternalOutput")

    cc_in = [nc.dram_tensor(f"cc_in{h}", [128 * NCORES, 256], BF)
             for h in range(3)]
    cc_out = [nc.dram_tensor(f"cc_out{h}", [128 * NCORES, 256], BF)
              for h in range(3)]

    with tile.TileContext(nc) as tc:
        with (
            tc.tile_pool(name="const", bufs=1) as cpool,
            tc.tile_pool(name="proj", bufs=1) as proj,
            tc.tile_pool(name="xstream", bufs=6) as xstream,
            tc.tile_pool(name="work", bufs=2) as work,
            tc.tile_pool(name="small", bufs=4) as small,
            tc.tile_pool(name="PL", bufs=2, space="PSUM") as PL,
            tc.tile_pool(name="PT", bufs=2, space="PSUM") as PT,
            tc.tile_pool(name="PX", bufs=1, space="PSUM") as PX,
            tc.tile_pool(name="PO", bufs=1, space="PSUM") as PO,
        ):
            # ---- constants
            wq_sb = cpool.tile([128, KT, 128], BF, tag="wq")
            wk_sb = cpool.tile([128, KT, 128], BF, tag="wk")
            wv_sb = cpool.tile([128, KT, 128], BF, tag="wv")
            fpe_sb = cpool.tile([128, KT, 128], BF, tag="fpe")
            for dst, srcw in ((wq_sb, wqT), (wk_sb, wkT), (wv_sb, wvT),
                              (fpe_sb, fpeT)):
                nc.gpsimd.dma_start(
                    dst[:], srcw.ap().rearrange("p (k c) -> p k c", k=KT))
            # wo (2MB) is only needed late: keep it off the startup HBM
            # burst so the first input chunks land sooner
            wo_sb = cpool.tile([128, KT, D], BF, tag="wo")
            with tc.tile_wait_until(0.1):
                nc.gpsimd.dma_start(
                    wo_sb[:], woT.ap().rearrange("p (k c) -> p k c", k=KT))
            ident = cpool.tile([128, 128], BF, tag="ident")
            nc.gpsimd.dma_start(ident[:], identc[:])

            # ---- persistent activations
            q2T = proj.tile([128, N], BF, tag="q2T")
            k2T = proj.tile([128, M], BF, tag="k2T")
            pe2T = proj.tile([128, RP], BF, tag="pe2T")
            v2 = proj.tile([128, NB, 128], BF, tag="v2")
            # exp'd positional logits, one per (block-in-pair, head)
            EP = [proj.tile([128, F2], BF, tag=f"ep{i}", name=f"ep{i}")
                  for i in range(4)]
            for i in range(4):
                nc.vector.memset(EP[i][:, RP:F2], 0.0)

            # ---- streamed projections -------------------------------
            def load_chunk(src, c, nm):
                t = xstream.tile([128, KT, CW], BF, tag="xs", name=nm)
                nc.sync.dma_start(
                    t[:],
                    src[:, c * KT * CW:(c + 1) * KT * CW]
                    .rearrange("p (k c) -> p k c", k=KT))
                return t

            def proj_cols(t, wtile, dest, c, nm):
                ps = PL.tile([128, 512], F32, tag="lg0", name=f"ps_{nm}")
                for k in range(KT):
                    nc.tensor.matmul(ps[:, :CW], wtile[:, k, :], t[:, k, :],
                                     start=(k == 0), stop=(k == KT - 1))
                nc.scalar.activation(dest[:, c * CW:(c + 1) * CW],
                                     ps[:, :CW], ACTF.Copy)

            def proj_v(t, c):
                for mt4 in range(CW // 128):
                    mt = c * (CW // 128) + mt4
                    ps = PL.tile([128, 512], F32, tag="lg1", name=f"psv{mt}")
                    for k in range(KT):
                        nc.tensor.matmul(
                            ps[:, :128], t[:, k, mt4 * 128:(mt4 + 1) * 128],
                            wv_sb[:, k, :],
                            start=(k == 0), stop=(k == KT - 1))
                    nc.vector.tensor_copy(v2[:, mt, :], ps[:, :128])

            def do_kv(c):
                t = load_chunk(xkvS, c, f"kv{c}")
                proj_cols(t, wk_sb, k2T, c, f"k{c}")
                proj_v(t, c)

            def do_q(c):
                t = load_chunk(xqS, c, f"q{c}")
                proj_cols(t, wq_sb, q2T, c, f"q{c}")

            def do_sc(c):
                t = load_chunk(scS, c, f"sc{c}")
                proj_cols(t, fpe_sb, pe2T, c, f"pe{c}")

            # ---- attention pair -------------------------------------
            def attn_S(j):
                pPs, dgs = {}, {}
                for gi, nb in ((0, 2 * j), (1, 2 * j + 1)):
                    n0 = nb * 128
                    span = n0 + 128
                    c_lo = (RP - 1 - n0 - 127) // 512
                    lo_col = RP - span   # first live column of plr
                    for hl in (0, 1):
                        hb = hl * 64
                        ep = EP[2 * gi + hl]
                        # positional logits, exp'd during PSUM drain
                        for cc in range(c_lo, 4):
                            w0 = max(cc * 512, lo_col)
                            w1 = (cc + 1) * 512
                            ps = PL.tile([128, 512], F32, tag=f"lg{hl}",
                                         name=f"plr{j}_{gi}_{hl}_{cc}")
                            nc.tensor.matmul(
                                ps[:, :w1 - w0],
                                q2T[hb:hb + 64, n0:n0 + 128],
                                pe2T[hb:hb + 64, w0:w1],
                                start=True, stop=True)
                            nc.scalar.activation(ep[:, w0:w1],
                                                 ps[:, :w1 - w0], ACTF.Exp)
                    for hl in (0, 1):
                        # skewed gather of exp'd positional logits (bf16);
                        # on the scalar HWDGE queue so collectives on the
                        # gpsimd queue never stall later pairs' gathers
                        sp = work.tile([128, span], BF, tag=f"sp{hl}",
                                       bufs=3, name=f"sp{j}_{gi}_{hl}")
                        ep = EP[2 * gi + hl]
                        skew = AP(ep[:].tensor,
                                  ep[:].offset + (RP - 1 - n0),
                                  [[F2 - 1, 128], [1, span]])
                        nc.sync.dma_start(sp[:], skew)
                        pPs[(gi, hl)] = (sp, None)
                    # content logits, exp'd during PSUM drain
                    sCs = {}
                    for hl in (0, 1):
                        sCs[hl] = work.tile([128, span], BF, tag=f"sc{hl}",
                                            name=f"sC{j}_{gi}_{hl}")
                    for ch in range((span + 511) // 512):
                        cw = min(512, span - ch * 512)
                        for hl in (0, 1):
                            hb = hl * 64
                            ps = PL.tile([128, 512], F32, tag=f"lg{hl}",
                                         name=f"cont{j}_{gi}_{hl}_{ch}")
                            nc.tensor.matmul(
                                ps[:, :cw],
                                q2T[hb:hb + 64, n0:n0 + 128],
                                k2T[hb:hb + 64, ch * 512:ch * 512 + cw],
                                start=True, stop=True)
                            nc.scalar.activation(
                                sCs[hl][:, ch * 512:ch * 512 + cw],
                                ps[:, :cw], ACTF.Exp)
                    # P = expC * expPos with fused row-sum; then diag(1/l)
                    for hl in (0, 1):
                        sp = pPs[(gi, hl)][0]
                        pP = work.tile([128, span], BF, tag=f"pp{hl}",
                                       bufs=4, name=f"pP{j}_{gi}_{hl}")
                        lrow = small.tile([128, 1], F32, tag=f"lr{gi}{hl}",
                                          bufs=4, name=f"lr{j}_{gi}_{hl}")
                        nc.vector.scalar_tensor_tensor(
                            pP[:], sCs[hl][:], 1.0, sp[:], MUL, MUL,
                            accum_out=lrow[:])
                        linv = small.tile([128, 1], F32, tag=f"li{gi}{hl}",
                                          bufs=4, name=f"li{j}_{gi}_{hl}")
                        nc.vector.reciprocal(linv[:], lrow[:])
                        dg = small.tile([128, 128], BF, tag=f"dg{gi}{hl}",
                                        bufs=4, name=f"dg{j}_{gi}_{hl}")
                        nc.vector.tensor_scalar_mul(dg[:], ident[:], linv[:])
                        pPs[(gi, hl)] = pP
                        dgs[(gi, hl)] = dg
                return pPs, dgs

            def attn_PV(j, pPs, dgs):
                na, nbt = 2 * j + 1, 2 * j + 2
                # transposes: per (mt, hl) group A[mt]|B[mt] -> [128,256];
                # two mt per psT bank -> one cast per 4 tiles
                ctxp = PX.tile([128, 256], F32, tag="ctx", name=f"ctx{j}")
                # phase 1: all transposes+casts, alternating heads so each
                # head's cast overlaps the other head's transposes (PT has
                # one bank per head); casts alternate DVE/ACT
                ptsbs = {0: [], 1: []}
                for g0 in range(0, nbt, 2):
                    for hl in (0, 1):
                        pt_ps = PT.tile([128, 512], F32, tag=f"ptT{hl}",
                                        bufs=1, name=f"ptps{j}_{hl}_{g0}")
                        for q, mt in enumerate(range(g0, min(g0 + 2, nbt))):
                            if mt < na:
                                nc.tensor.matmul(
                                    pt_ps[:, q * 256:q * 256 + 128],
                                    pPs[(0, hl)][:, mt * 128:(mt + 1) * 128],
                                    dgs[(0, hl)][:], start=True, stop=True)
                            nc.tensor.matmul(
                                pt_ps[:, q * 256 + 128:q * 256 + 256],
                                pPs[(1, hl)][:, mt * 128:(mt + 1) * 128],
                                dgs[(1, hl)][:], start=True, stop=True)
                        w = min(2, nbt - g0) * 256
                        pt_sb = small.tile([128, 512], BF, tag=f"ptsb{hl}",
                                           bufs=8, name=f"ptsb{j}_{hl}_{g0}")
                        if (g0 // 2 + hl) % 3 != 2:
                            nc.vector.tensor_copy(pt_sb[:, :w], pt_ps[:, :w])
                        else:
                            nc.scalar.activation(pt_sb[:, :w], pt_ps[:, :w],
                                                 ACTF.Copy)
                        ptsbs[hl].append(pt_sb)
                # phase 2: ctx matmuls, head-serial per has_written rules
                for hl in (0, 1):
                    ob = hl * 64
                    for g0 in range(0, nbt, 2):
                        pt_sb = ptsbs[hl][g0 // 2]
                        for q, mt in enumerate(range(g0, min(g0 + 2, nbt))):
                            if mt < na:
                                nc.tensor.matmul(
                                    ctxp[ob:ob + 64, :],
                                    v2[:, mt, hl * 64:hl * 64 + 64],
                                    pt_sb[:, q * 256:(q + 1) * 256],
                                    start=(mt == 0),
                                    stop=(mt == nbt - 1))
                            else:
                                nc.tensor.matmul(
                                    ctxp[ob:ob + 64, 128:256],
                                    v2[:, mt, hl * 64:hl * 64 + 64],
                                    pt_sb[:, q * 256 + 128:q * 256 + 256],
                                    start=False, stop=(mt == nbt - 1))
                # ship pair-j ctx (dvh 128 x n 256) to dest core j
                ctxs = work.tile([128, 256], BF, tag="ship",
                                 name=f"ship{j}")
                # pair 7's ship gates the final collective's rendezvous on
                # every core: route it through ACT, which is idle by then,
                # instead of the still-backlogged DVE
                if j == 7:
                    nc.scalar.activation(ctxs[:], ctxp[:], ACTF.Copy)
                else:
                    nc.vector.tensor_copy(ctxs[:], ctxp[:])
                cc_t = cc_in[0] if j < 4 else cc_in[1]
                nc.sync.dma_start(cc_t[j * 128:(j + 1) * 128, :], ctxs[:])

            def a2a(half):
                nc.gpsimd.collective_compute(
                    "AllToAll",
                    mybir.AluOpType.bypass,
                    ins=[cc_in[half][:]],
                    outs=[cc_out[half][:]],
                    replica_groups=RG,
                )

            def load_stages(pass_id):
                # pass A loads on gpsimd (hidden under attention); later
                # passes on the by-then-idle sync ring (lower latency).
                # 8 separate contiguous loads: they all wait on the same
                # collective and then run concurrently across SDMA lanes
                dma_eng = nc.gpsimd if pass_id == 0 else nc.sync
                src = cc_out[pass_id]
                stages = [small.tile([128, 256], BF, tag=f"st{k}", bufs=1,
                                     name=f"st{pass_id}_{k}")
                          for k in range(KT)]
                for k in range(KT):
                    dma_eng.dma_start(stages[k][:],
                                      src[k * 128:(k + 1) * 128, :])
                return stages

            def outproj(pass_id, row0, stages):
                dma_eng = nc.gpsimd if pass_id == 0 else nc.sync
                for nh in (0, 1):
                    for dc in (0, 1):
                        # alternate PSUM banks (PT is free this late) so
                        # chain k+1 never waits chain k's ACT drain
                        if (nh + dc) % 2 == 0:
                            ps = PO.tile([128, 512], F32, tag="out",
                                         name=f"o{pass_id}_{nh}_{dc}")
                        else:
                            ps = PT.tile([128, 512], F32, tag="ptT1",
                                         bufs=1, name=f"o{pass_id}_{nh}_{dc}")
                        for k in range(KT):
                            nc.tensor.matmul(
                                ps[:],
                                stages[k][:, nh * 128:(nh + 1) * 128],
                                wo_sb[:, k, dc * 512:(dc + 1) * 512],
                                start=(k == 0), stop=(k == KT - 1))
                        ost = small.tile([128, 512], F32, tag="ostage",
                                         bufs=2, name=f"os{pass_id}{nh}{dc}")
                        nc.scalar.activation(ost[:], ps[:], ACTF.Copy)
                        dma_eng.dma_start(
                            out_ext[row0 + nh * 128:row0 + nh * 128 + 128,
                                    dc * 512:(dc + 1) * 512], ost[:])

            # ---- schedule: software-pipelined S (logits/softmax, dense
            # 512-wide matmuls) against PV (LDW-heavy transposes+ctx) of
            # the previous pair, keeping PE array activity high
            for j in range(8):
                do_kv(j)
                do_q(j)
                do_sc(7 - j)
                if j == 4:
                    a2a(0)
                attn_PV(j, *attn_S(j))
            # pass A's data (A2A#0) has long landed: issue it BEFORE the
            # second collective so its gpsimd-ring DMAs are not queued
            # behind the collective wait, and its matmuls run during
            # A2A#1's rendezvous (doubling as the HAM warm-keeper);
            # tile_wait_until pins these AFTER the attention pairs in the
            # static schedule (else the scheduler hoists the
            # collective-gated matmuls into the middle of the PE queue,
            # stalling attention on the rendezvous)
            with tc.tile_wait_until(0.12):
                stA = load_stages(0)
            with tc.tile_wait_until(0.178):
                outproj(0, 0, stA)
            a2a(1)
            with tc.tile_wait_until(0.185):
                warm1 = PT.tile([128, 512], F32, tag="ptT0", bufs=1,
                                name="warmB")
                # 28 x ~216ns ~= 6us: long enough to cross the ~3.4us HAM
                # un-throttle window and to span most of the collective
                # wait, so pass-B matmuls run at the warm clock
                for wi in range(44):
                    nc.tensor.matmul(warm1[:], wo_sb[:, 0, 0:128],
                                     wo_sb[:, 0, 0:512], start=True,
                                     stop=True)
            with tc.tile_wait_until(0.19):
                stB = load_stages(1)
                outproj(1, 256, stB)

    nc.compile()
    return nc


def _host_prep(inputs):
    bf16 = ml_dtypes.bfloat16
    x_q = np.asarray(inputs["x_q"])[:, 0, :]
    x_kv = np.asarray(inputs["x_kv"])[:, 0, :]
    to_q = np.asarray(inputs["to_q"])
    to_k = np.asarray(inputs["to_k"])
    to_v = np.asarray(inputs["to_v"])
    to_out = np.asarray(inputs["to_out"])
    fpe = np.asarray(inputs["for_pos_enc"])

    r = np.arange(0, RP, dtype=np.float32)
    inv_freq = 1.0 / (10000.0 ** (np.arange(0.0, D, 2.0, np.float32) / D))
    ph = r[:, None] * inv_freq[None, :]
    sincos = np.concatenate([np.sin(ph), np.cos(ph)], axis=-1)
    scT = np.ascontiguousarray(sincos[::-1].T)  # [D, RP]

    def chunked(xT):
        # xT [D, N] -> [128, NCH*KT*CW]: chunk-contiguous [p, c, k, j]
        a = xT.reshape(KT, 128, NCH, CW).transpose(1, 2, 0, 3)
        return np.ascontiguousarray(a.reshape(128, NCH * KT * CW)).astype(
            bf16)

    xqS = chunked(x_q.T)
    xkvS = chunked(x_kv.T)
    scS = chunked(scT)

    wo_ckd = (to_out.transpose(0, 2, 1).reshape(D, H * DV).T
              .reshape(KT, 128, D).transpose(1, 0, 2).reshape(128, KT * D))
    woT = np.ascontiguousarray(wo_ckd).astype(bf16)
    identity = np.eye(128, dtype=bf16)

    def shuf(w):
        return np.ascontiguousarray(
            w.reshape(KT, 128, 128).transpose(1, 0, 2).reshape(128, KT * 128)
        ).astype(bf16)

    in_maps = []
    for c in range(NCORES):
        hs = [2 * c, 2 * c + 1]
        in_maps.append({
            "xqS": xqS, "xkvS": xkvS, "scS": scS,
            "wqT": shuf(np.concatenate([to_q[:, h, :].T for h in hs], 1)),
            "wkT": shuf(np.concatenate([to_k[:, h, :].T for h in hs], 1)),
            "wvT": shuf(np.concatenate([to_v[:, h, :].T for h in hs], 1)),
            "fpeT": shuf(np.concatenate([fpe[:, h, :].T for h in hs], 1)),
            "woT": woT, "identc": identity,
        })
    return in_maps


def _gather(res):
    def region(c):
        return 0 if c < 4 else 256
    rows = [res.results[c]["out"][region(c):region(c) + 256]
            for c in range(NCORES)]
    out = np.concatenate(rows, 0)
    return out.reshape(N, 1, D).astype(np.float32)


def kernel(**inputs):
    if "nc" not in _cache:
        _cache["nc"] = _build()
    nc = _cache["nc"]
    in_maps = _host_prep(inputs)
    res = run_bass_kernel_spmd(nc, in_maps, list(range(NCORES)))
    return _gather(res)


if __name__ == "__main__":
    import pickle
    with open("/tmp/inputs.pkl", "rb") as f:
        inputs = pickle.load(f)
    out = kernel(**inputs)
    exp = np.load("/tmp/expected.npy")
    err = np.linalg.norm(out - exp) / np.linalg.norm(exp)
    print("Relative error:", err)


# revision 81
# speedup vs baseline: 1.0165x; 1.0165x over previous
"""Distributed Trainium2 kernel for relative-position causal attention.

N=M=2048, B=1, D=1024, H=16, DQK=DV=64, OFFSET=0.
2 heads per core on 8 NeuronCores.

v2 structure (vs baseline):
  - inputs streamed column-chunked (1MB contiguous DMAs); projections
    interleave with attention so pair 0 starts ~10us in
  - scheme C softmax: exp(plr) and exp(content) separately on ACT
    (the PSUM drain IS the exp), position gathered post-exp as bf16 via
    the diagonal skew DMA, P = expC*expPos fused with the row-sum via
    scalar_tensor_tensor on DVE (one pass, accum_out = l)
  - logit matmuls head-paired: q/k/pe slices at partitions 0-63 vs
    64-127 auto-derive tile_position row groups -> concurrent on PE
  - ctx matmuls col-paired across the two heads (out partitions 0-63 /
    64-127)
  - transposes via diag(1/l) matmul (normalize+transpose fused), psT
    packs 4 P^T tiles per PSUM bank, single cast per 4 tiles
  - tail: two AllToAlls regrouped as pairs 0-3 / 4-7 (A2A#0 fires
    mid-kernel and its transfer hides under pairs 4-7); out-projection
    runs twice -- pass A executes during A2A#1's rendezvous (doubling
    as the HAM warm-keeper), pass B is the only serial tail; the host
    picks rows [0:256] on cores 0-3 and [256:512] on cores 4-7.
  - tile_wait_until pins collective-gated work at the end of the static
    schedule; queue split: sync ring = input chunks + skews + ships,
    scalar ring = compute only, gpsimd ring = weights + collectives +
    pass-A DMA.
"""

import sys

sys.path.insert(0, "/opt/trn_rl_repo")

import numpy as np
import ml_dtypes

from concourse import bass, bacc, tile, mybir
from concourse.ap import AP
from concourse.bass_utils import run_bass_kernel_spmd

N, M, D, H, DQK, DV = 2048, 2048, 1024, 16, 64, 64
RP = 2048
NCORES = 8
NB = N // 128
KT = D // 128          # 8 contraction k-tiles
F2 = RP + 128
CW = 256               # streamed input chunk width
NCH = N // CW          # 8 chunks per input tensor

BF = mybir.dt.bfloat16
F32 = mybir.dt.float32
RG = [list(range(NCORES))]
_cache = {}


def _build():
    nc = bacc.Bacc("TRN2", target_bir_lowering=False, debug=False,
                   num_devices=NCORES)
    ACTF = mybir.ActivationFunctionType
    MUL = mybir.AluOpType.mult

    # chunked inputs: [128, NCH*KT*CW], chunk c contiguous as [128,KT,CW]
    xqS = nc.dram_tensor("xqS", [128, NCH * KT * CW], BF, kind="ExternalInput")
    xkvS = nc.dram_tensor("xkvS", [128, NCH * KT * CW], BF,
                          kind="ExternalInput")
    scS = nc.dram_tensor("scS", [128, NCH * KT * CW], BF, kind="ExternalInput")
    wqT = nc.dram_tensor("wqT", [128, KT * 128], BF, kind="ExternalInput")
    wkT = nc.dram_tensor("wkT", [128, KT * 128], BF, kind="ExternalInput")
    wvT = nc.dram_tensor("wvT", [128, KT * 128], BF, kind="ExternalInput")
    fpeT = nc.dram_tensor("fpeT", [128, KT * 128], BF, kind="ExternalInput")
    woT = nc.dram_tensor("woT", [128, KT * D], BF, kind="ExternalInput")
    identc = nc.dram_tensor("identc", [128, 128], BF, kind="ExternalInput")
    out_ext = nc.dram_tensor("out", [512, D], F32, kind="ExternalOutput")

    cc_in = [nc.dram_tensor(f"cc_in{h}", [128 * NCORES, 256], BF)
             for h in range(3)]
    cc_out = [nc.dram_tensor(f"cc_out{h}", [128 * NCORES, 256], BF)
              for h in range(3)]

    with tile.TileContext(nc) as tc:
        with (
            tc.tile_pool(name="const", bufs=1) as cpool,
            tc.tile_pool(name="proj", bufs=1) as proj,
            tc.tile_pool(name="xstream", bufs=6) as xstream,
            tc.tile_pool(name="work", bufs=2) as work,
            tc.tile_pool(name="small", bufs=4) as small,
            tc.tile_pool(name="PL", bufs=2, space="PSUM") as PL,
            tc.tile_pool(name="PT", bufs=2, space="PSUM") as PT,
            tc.tile_pool(name="PX", bufs=1, space="PSUM") as PX,
            tc.tile_pool(name="PO", bufs=1, space="PSUM") as PO,
        ):
            # ---- constants
            wq_sb = cpool.tile([128, KT, 128], BF, tag="wq")
            wk_sb = cpool.tile([128, KT, 128], BF, tag="wk")
            wv_sb = cpool.tile([128, KT, 128], BF, tag="wv")
            fpe_sb = cpool.tile([128, KT, 128], BF, tag="fpe")
            for dst, srcw in ((wq_sb, wqT), (wk_sb, wkT), (wv_sb, wvT),
                              (fpe_sb, fpeT)):
                nc.gpsimd.dma_start(
                    dst[:], srcw.ap().rearrange("p (k c) -> p k c", k=KT))
            # wo (2MB) is only needed late: keep it off the startup HBM
            # burst so the first input chunks land sooner
            wo_sb = cpool.tile([128, KT, D], BF, tag="wo")
            with tc.tile_wait_until(0.1):
                nc.gpsimd.dma_start(
                    wo_sb[:], woT.ap().rearrange("p (k c) -> p k c", k=KT))
            ident = cpool.tile([128, 128], BF, tag="ident")
            nc.gpsimd.dma_start(ident[:], identc[:])

            # ---- persistent activations
            q2T = proj.tile([128, N], BF, tag="q2T")
            k2T = proj.tile([128, M], BF, tag="k2T")
            pe2T = proj.tile([128, RP], BF, tag="pe2T")
            v2 = proj.tile([128, NB, 128], BF, tag="v2")
            # exp'd positional logits, one per (block-in-pair, head)
            EP = [proj.tile([128, F2], BF, tag=f"ep{i}", name=f"ep{i}")
                  for i in range(4)]
            for i in range(4):
                nc.vector.memset(EP[i][:, RP:F2], 0.0)

            # ---- streamed projections -------------------------------
            def load_chunk(src, c, nm):
                t = xstream.tile([128, KT, CW], BF, tag="xs", name=nm)
                nc.sync.dma_start(
                    t[:],
                    src[:, c * KT * CW:(c + 1) * KT * CW]
                    .rearrange("p (k c) -> p k c", k=KT))
                return t

            def proj_cols(t, wtile, dest, c, nm):
                ps = PL.tile([128, 512], F32, tag="lg0", name=f"ps_{nm}")
                for k in range(KT):
                    nc.tensor.matmul(ps[:, :CW], wtile[:, k, :], t[:, k, :],
                                     start=(k == 0), stop=(k == KT - 1))
                nc.scalar.activation(dest[:, c * CW:(c + 1) * CW],
                                     ps[:, :CW], ACTF.Copy)

            def proj_v(t, c):
                for mt4 in range(CW // 128):
                    mt = c * (CW // 128) + mt4
                    ps = PL.tile([128, 512], F32, tag="lg1", name=f"psv{mt}")
                    for k in range(KT):
                        nc.tensor.matmul(
                            ps[:, :128], t[:, k, mt4 * 128:(mt4 + 1) * 128],
                            wv_sb[:, k, :],
                            start=(k == 0), stop=(k == KT - 1))
                    nc.vector.tensor_copy(v2[:, mt, :], ps[:, :128])

            def do_kv(c):
                t = load_chunk(xkvS, c, f"kv{c}")
                proj_cols(t, wk_sb, k2T, c, f"k{c}")
                proj_v(t, c)

            def do_q(c):
                t = load_chunk(xqS, c, f"q{c}")
                proj_cols(t, wq_sb, q2T, c, f"q{c}")

            def do_sc(c):
                t = load_chunk(scS, c, f"sc{c}")
                proj_cols(t, fpe_sb, pe2T, c, f"pe{c}")

            # ---- attention pair -------------------------------------
            def attn_S(j):
                pPs, dgs = {}, {}
                for gi, nb in ((0, 2 * j), (1, 2 * j + 1)):
                    n0 = nb * 128
                    span = n0 + 128
                    c_lo = (RP - 1 - n0 - 127) // 512
                    lo_col = RP - span   # first live column of plr
                    for hl in (0, 1):
                        hb = hl * 64
                        ep = EP[2 * gi + hl]
                        # positional logits, exp'd during PSUM drain
                        for cc in range(c_lo, 4):
                            w0 = max(cc * 512, lo_col)
                            w1 = (cc + 1) * 512
                            ps = PL.tile([128, 512], F32, tag=f"lg{hl}",
                                         name=f"plr{j}_{gi}_{hl}_{cc}")
                            nc.tensor.matmul(
                                ps[:, :w1 - w0],
                                q2T[hb:hb + 64, n0:n0 + 128],
                                pe2T[hb:hb + 64, w0:w1],
                                start=True, stop=True)
                            nc.scalar.activation(ep[:, w0:w1],
                                                 ps[:, :w1 - w0], ACTF.Exp)
                    for hl in (0, 1):
                        # skewed gather of exp'd positional logits (bf16);
                        # on the scalar HWDGE queue so collectives on the
                        # gpsimd queue never stall later pairs' gathers
                        sp = work.tile([128, span], BF, tag=f"sp{hl}",
                                       bufs=3, name=f"sp{j}_{gi}_{hl}")
                        ep = EP[2 * gi + hl]
                        skew = AP(ep[:].tensor,
                                  ep[:].offset + (RP - 1 - n0),
                                  [[F2 - 1, 128], [1, span]])
                        nc.sync.dma_start(sp[:], skew)
                        pPs[(gi, hl)] = (sp, None)
                    # content logits, exp'd during PSUM drain
                    sCs = {}
                    for hl in (0, 1):
                        sCs[hl] = work.tile([128, span], BF, tag=f"sc{hl}",
                                            name=f"sC{j}_{gi}_{hl}")
                    for ch in range((span + 511) // 512):
                        cw = min(512, span - ch * 512)
                        for hl in (0, 1):
                            hb = hl * 64
                            ps = PL.tile([128, 512], F32, tag=f"lg{hl}",
                                         name=f"cont{j}_{gi}_{hl}_{ch}")
                            nc.tensor.matmul(
                                ps[:, :cw],
                                q2T[hb:hb + 64, n0:n0 + 128],
                                k2T[hb:hb + 64, ch * 512:ch * 512 + cw],
                                start=True, stop=True)
                            nc.scalar.activation(
                                sCs[hl][:, ch * 512:ch * 512 + cw],
                                ps[:, :cw], ACTF.Exp)
                    # P = expC * expPos with fused row-sum; then diag(1/l)
                    for hl in (0, 1):
                        sp = pPs[(gi, hl)][0]
                        pP = work.tile([128, span], BF, tag=f"pp{hl}",
                                       bufs=4, name=f"pP{j}_{gi}_{hl}")
                        lrow = small.tile([128, 1], F32, tag=f"lr{gi}{hl}",
                                          bufs=4, name=f"lr{j}_{gi}_{hl}")
                        nc.vector.scalar_tensor_tensor(
                            pP[:], sCs[hl][:], 1.0, sp[:], MUL, MUL,
                            accum_out=lrow[:])
                        linv = small.tile([128, 1], F32, tag=f"li{gi}{hl}",
                                          bufs=4, name=f"li{j}_{gi}_{hl}")
                        nc.vector.reciprocal(linv[:], lrow[:])
                        dg = small.tile([128, 128], BF, tag=f"dg{gi}{hl}",
                                        bufs=4, name=f"dg{j}_{gi}_{hl}")
                        nc.vector.tensor_scalar_mul(dg[:], ident[:], linv[:])
                        pPs[(gi, hl)] = pP
                        dgs[(gi, hl)] = dg
                return pPs, dgs

            def attn_PV(j, pPs, dgs):
                na, nbt = 2 * j + 1, 2 * j + 2
                # transposes: per (mt, hl) group A[mt]|B[mt] -> [128,256];
                # two mt per psT bank -> one cast per 4 tiles
                ctxp = PX.tile([128, 256], F32, tag="ctx", name=f"ctx{j}")
                # phase 1: all transposes+casts, alternating heads so each
                # head's cast overlaps the other head's transposes (PT has
                # one bank per head); casts alternate DVE/ACT
                ptsbs = {0: [], 1: []}
                for g0 in range(0, nbt, 2):
                    for hl in (0, 1):
                        pt_ps = PT.tile([128, 512], F32, tag=f"ptT{hl}",
                                        bufs=1, name=f"ptps{j}_{hl}_{g0}")
                        for q, mt in enumerate(range(g0, min(g0 + 2, nbt))):
                            if mt < na:
                                nc.tensor.matmul(
                                    pt_ps[:, q * 256:q * 256 + 128],
                                    pPs[(0, hl)][:, mt * 128:(mt + 1) * 128],
                                    dgs[(0, hl)][:], start=True, stop=True)
                            nc.tensor.matmul(
                                pt_ps[:, q * 256 + 128:q * 256 + 256],
                                pPs[(1, hl)][:, mt * 128:(mt + 1) * 128],
                                dgs[(1, hl)][:], start=True, stop=True)
                        w = min(2, nbt - g0) * 256
                        pt_sb = small.tile([128, 512], BF, tag=f"ptsb{hl}",
                                           bufs=8, name=f"ptsb{j}_{hl}_{g0}")
                        if (g0 // 2 + hl) % 3 != 2:
                            nc.vector.tensor_copy(pt_sb[:, :w], pt_ps[:, :w])
                        else:
                            nc.scalar.activation(pt_sb[:, :w], pt_ps[:, :w],
                                                 ACTF.Copy)
                        ptsbs[hl].append(pt_sb)
                # phase 2: ctx matmuls, head-serial per has_written rules
                for hl in (0, 1):
                    ob = hl * 64
                    for g0 in range(0, nbt, 2):
                        pt_sb = ptsbs[hl][g0 // 2]
                        for q, mt in enumerate(range(g0, min(g0 + 2, nbt))):
                            if mt < na:
                                nc.tensor.matmul(
                                    ctxp[ob:ob + 64, :],
                                    v2[:, mt, hl * 64:hl * 64 + 64],
                                    pt_sb[:, q * 256:(q + 1) * 256],
                                    start=(mt == 0),
                                    stop=(mt == nbt - 1))
                            else:
                                nc.tensor.matmul(
                                    ctxp[ob:ob + 64, 128:256],
                                    v2[:, mt, hl * 64:hl * 64 + 64],
                                    pt_sb[:, q * 256 + 128:q * 256 + 256],
                                    start=False, stop=(mt == nbt - 1))
                # ship pair-j ctx (dvh 128 x n 256) to dest core j
                ctxs = work.tile([128, 256], BF, tag="ship",
                                 name=f"ship{j}")
                nc.vector.tensor_copy(ctxs[:], ctxp[:])
                cc_t = cc_in[0] if j < 4 else cc_in[1]
                nc.sync.dma_start(cc_t[j * 128:(j + 1) * 128, :], ctxs[:])

            def a2a(half):
                nc.gpsimd.collective_compute(
                    "AllToAll",
                    mybir.AluOpType.bypass,
                    ins=[cc_in[half][:]],
                    outs=[cc_out[half][:]],
                    replica_groups=RG,
                )

            def load_stages(pass_id):
                # pass A loads on gpsimd (hidden under attention); later
                # passes on the by-then-idle sync ring (lower latency).
                # 8 separate contiguous loads: they all wait on the same
                # collective and then run concurrently across SDMA lanes
                dma_eng = nc.gpsimd if pass_id == 0 else nc.sync
                src = cc_out[pass_id]
                stages = [small.tile([128, 256], BF, tag=f"st{k}", bufs=1,
                                     name=f"st{pass_id}_{k}")
                          for k in range(KT)]
                for k in range(KT):
                    dma_eng.dma_start(stages[k][:],
                                      src[k * 128:(k + 1) * 128, :])
                return stages

            def outproj(pass_id, row0, stages):
                dma_eng = nc.gpsimd if pass_id == 0 else nc.sync
                for nh in (0, 1):
                    for dc in (0, 1):
                        # alternate PSUM banks (PT is free this late) so
                        # chain k+1 never waits chain k's ACT drain
                        if (nh + dc) % 2 == 0:
                            ps = PO.tile([128, 512], F32, tag="out",
                                         name=f"o{pass_id}_{nh}_{dc}")
                        else:
                            ps = PT.tile([128, 512], F32, tag="ptT1",
                                         bufs=1, name=f"o{pass_id}_{nh}_{dc}")
                        for k in range(KT):
                            nc.tensor.matmul(
                                ps[:],
                                stages[k][:, nh * 128:(nh + 1) * 128],
                                wo_sb[:, k, dc * 512:(dc + 1) * 512],
                                start=(k == 0), stop=(k == KT - 1))
                        ost = small.tile([128, 512], F32, tag="ostage",
                                         bufs=2, name=f"os{pass_id}{nh}{dc}")
                        nc.scalar.activation(ost[:], ps[:], ACTF.Copy)
                        dma_eng.dma_start(
                            out_ext[row0 + nh * 128:row0 + nh * 128 + 128,
                                    dc * 512:(dc + 1) * 512], ost[:])

            # ---- schedule: software-pipelined S (logits/softmax, dense
            # 512-wide matmuls) against PV (LDW-heavy transposes+ctx) of
            # the previous pair, keeping PE array activity high
            for j in range(8):
                do_kv(j)
                do_q(j)
                do_sc(7 - j)
                if j == 4:
                    a2a(0)
                attn_PV(j, *attn_S(j))
            # pass A's data (A2A#0) has long landed: issue it BEFORE the
            # second collective so its gpsimd-ring DMAs are not queued
            # behind the collective wait, and its matmuls run during
            # A2A#1's rendezvous (doubling as the HAM warm-keeper);
            # tile_wait_until pins these AFTER the attention pairs in the
            # static schedule (else the scheduler hoists the
            # collective-gated matmuls into the middle of the PE queue,
            # stalling attention on the rendezvous)
            with tc.tile_wait_until(0.12):
                stA = load_stages(0)
            with tc.tile_wait_until(0.178):
                outproj(0, 0, stA)
            a2a(1)
            with tc.tile_wait_until(0.185):
                warm1 = PT.tile([128, 512], F32, tag="ptT0", bufs=1,
                                name="warmB")
                # 28 x ~216ns ~= 6us: long enough to cross the ~3.4us HAM
                # un-throttle window and to span most of the collective
                # wait, so pass-B matmuls run at the warm clock
                for wi in range(44):
                    nc.tensor.matmul(warm1[:], wo_sb[:, 0, 0:128],
                                     wo_sb[:, 0, 0:512], start=True,
                                     stop=True)
            with tc.tile_wait_until(0.19):
                stB = load_stages(1)
                outproj(1, 256, stB)

    nc.compile()
    return nc


def _host_prep(inputs):
    bf16 = ml_dtypes.bfloat16
    x_q = np.asarray(inputs["x_q"])[:, 0, :]
    x_kv = np.asarray(inputs["x_kv"])[:, 0, :]
    to_q = np.asarray(inputs["to_q"])
    to_k = np.asarray(inputs["to_k"])
    to_v = np.asarray(inputs["to_v"])
    to_out = np.asarray(inputs["to_out"])
    fpe = np.asarray(inputs["for_pos_enc"])

    r = np.arange(0, RP, dtype=np.float32)
    inv_freq = 1.0 / (10000.0 ** (np.arange(0.0, D, 2.0, np.float32) / D))
    ph = r[:, None] * inv_freq[None, :]
    sincos = np.concatenate([np.sin(ph), np.cos(ph)], axis=-1)
    scT = np.ascontiguousarray(sincos[::-1].T)  # [D, RP]

    def chunked(xT):
        # xT [D, N] -> [128, NCH*KT*CW]: chunk-contiguous [p, c, k, j]
        a = xT.reshape(KT, 128, NCH, CW).transpose(1, 2, 0, 3)
        return np.ascontiguousarray(a.reshape(128, NCH * KT * CW)).astype(
            bf16)

    xqS = chunked(x_q.T)
    xkvS = chunked(x_kv.T)
    scS = chunked(scT)

    wo_ckd = (to_out.transpose(0, 2, 1).reshape(D, H * DV).T
              .reshape(KT, 128, D).transpose(1, 0, 2).reshape(128, KT * D))
    woT = np.ascontiguousarray(wo_ckd).astype(bf16)
    identity = np.eye(128, dtype=bf16)

    def shuf(w):
        return np.ascontiguousarray(
            w.reshape(KT, 128, 128).transpose(1, 0, 2).reshape(128, KT * 128)
        ).astype(bf16)

    in_maps = []
    for c in range(NCORES):
        hs = [2 * c, 2 * c + 1]
        in_maps.append({
            "xqS": xqS, "xkvS": xkvS, "scS": scS,
            "wqT": shuf(np.concatenate([to_q[:, h, :].T for h in hs], 1)),
            "wkT": shuf(np.concatenate([to_k[:, h, :].T for h in hs], 1)),
            "wvT": shuf(np.concatenate([to_v[:, h, :].T for h in hs], 1)),
            "fpeT": shuf(np.concatenate([fpe[:, h, :].T for h in hs], 1)),
            "woT": woT, "identc": identity,
        })
    return in_maps


def _gather(res):
    def region(c):
        return 0 if c < 4 else 256
    rows = [res.results[c]["out"][region(c):region(c) + 256]
            for c in range(NCORES)]
    out = np.concatenate(rows, 0)
    return out.reshape(N, 1, D).astype(np.float32)


def kernel(**inputs):
    if "nc" not in _cache:
        _cache["nc"] = _build()
    nc = _cache["nc"]
    in_maps = _host_prep(inputs)
    res = run_bass_kernel_spmd(nc, in_maps, list(range(NCORES)))
    return _gather(res)


if __name__ == "__main__":
    import pickle
    with open("/tmp/inputs.pkl", "rb") as f:
        inputs = pickle.load(f)
    out = kernel(**inputs)
    exp = np.load("/tmp/expected.npy")
    err = np.linalg.norm(out - exp) / np.linalg.norm(exp)
    print("Relative error:", err)
